# revision 6
# baseline (speedup 1.0000x reference)
"""Cross-attention Trainium2 kernel (nn_CrossAttention_8486855377137).

Sharding (8 cores): core c = (batch b = c//2, head-group g = c%2).
Each core handles one batch and 4 of the 8 heads (Q/K/V projections
column-sharded by head, wo row-sharded). Full softmax over S on device;
host sums the two partial wo outputs per batch and adds wo bias.

v3: ACT-bound fused pipeline.
  - ScalarE exp stream (33.5M elem/core ~= 284us) is the roofline; the
    whole schedule exists to keep it dense from ~15us onward.
  - K path in fp8e4 DoubleRow (ctx fp8 + wk fp8 x512, scale folded into
    the exp activation scale). V path + es stay bf16 (fp8 there costs
    ~1e-2 rel err each, too close to the 2e-2 gate).
  - scores: bf16, two heads concurrent on PE row-groups 0:64/64:128.
  - K/V production is interleaved with the first TWO attention units
    (generator-based chase, half-s-group granularity) so ACT never
    starves; those units defer attnV one half-group behind exp.
  - Softmax denominator via ones-column in V; reciprocal_approx_fast.
"""

import numpy as np
import ml_dtypes

import concourse.bass as bass
import concourse.bacc as bacc
import concourse.tile as tile
import concourse.mybir as mybir
from concourse.bass_utils import run_bass_kernel_spmd

BF16 = mybir.dt.bfloat16
F32 = mybir.dt.float32
FP8 = mybir.dt.float8e4
EXP = mybir.ActivationFunctionType.Exp
ADD = mybir.AluOpType.add
MULT = mybir.AluOpType.mult
DR = mybir.MatmulPerfMode.DoubleRow
NPBF16 = ml_dtypes.bfloat16
NPFP8 = ml_dtypes.float8_e4m3

# Problem constants (hardcoded per contract)
B, T, S = 4, 2048, 4096
E, KV = 512, 2048
H, D = 8, 64
GE = 256            # head-group embed width (4 heads x 64)
SCALE = D ** -0.5   # 0.125
WK_SCALE = 512.0    # wk prescale (fp8 denormal avoidance); folded into exp scale
EXP_BIAS = -1.5     # exp(x + bias): cancels in softmax, shrinks es range

N_CORES = 8
P = 128
NT = T // 512       # 4 t-chunks
NSC = S // P        # 32 s-tiles
SGT = 4             # s-tiles per chase production step (512 cols)
NSG = NSC // SGT    # 8 production steps
KV_C = KV // P      # 16 contraction chunks for K/V proj
E_C = E // P        # 4 contraction chunks for Q proj


def _build_nc():
    nc = bacc.Bacc("TRN2", target_bir_lowering=False, debug=False)

    ctx8 = nc.dram_tensor("ctx8", [KV, S], FP8, kind="ExternalInput")
    xT = nc.dram_tensor("xT", [E, T], BF16, kind="ExternalInput")
    wqT = nc.dram_tensor("wqT", [E, GE], BF16, kind="ExternalInput")
    wkT = nc.dram_tensor("wkT", [KV, GE], FP8, kind="ExternalInput")
    wvT = nc.dram_tensor("wvT", [KV, GE], BF16, kind="ExternalInput")
    woT = nc.dram_tensor("woT", [GE, E], BF16, kind="ExternalInput")
    bq = nc.dram_tensor("bq", [GE], F32, kind="ExternalInput")
    bk = nc.dram_tensor("bk", [GE], F32, kind="ExternalInput")   # x512
    bv = nc.dram_tensor("bv", [GE], F32, kind="ExternalInput")
    yT = nc.dram_tensor("yT", [E, T], F32, kind="ExternalOutput")

    with tile.TileContext(nc) as tc:
        _kernel_body(tc, nc, ctx8, xT, wqT, wkT, wvT, woT, bq, bk, bv, yT)
    nc.compile()
    return nc


def _kernel_body(tc, nc, ctx8, xT, wqT, wkT, wvT, woT, bq, bk, bv, yT):
    wts = tc.alloc_tile_pool(name="wts", bufs=1)
    persist = tc.alloc_tile_pool(name="persist", bufs=1)

    # ---- constant / weight loads (order = DMA queue order: K path + Q first) ----
    wkT_sb = wts.tile([P, KV_C, GE], FP8, tag="wkT")
    nc.sync.dma_start(wkT_sb, wkT.rearrange("(c p) m -> p c m", p=P))
    wqT_sb = wts.tile([P, E_C, GE], BF16, tag="wqT")
    nc.sync.dma_start(wqT_sb, wqT.rearrange("(c p) m -> p c m", p=P))
    xT_sb = wts.tile([P, E_C, T], BF16, tag="xT")
    nc.sync.dma_start(xT_sb, xT.rearrange("(c p) t -> p c t", p=P))
    wvT_sb = wts.tile([P, KV_C, GE], BF16, tag="wvT")
    nc.sync.dma_start(wvT_sb, wvT.rearrange("(c p) m -> p c m", p=P))
    woT_sb = wts.tile([P, 2, E], BF16, tag="woT")
    nc.sync.dma_start(woT_sb, woT.rearrange("(c p) m -> p c m", p=P))

    bq_sb = wts.tile([P, 2], F32, tag="bq")
    nc.sync.dma_start(bq_sb, bq.rearrange("(c p) -> p c", p=P))
    bk_sb = wts.tile([P, 2], F32, tag="bk")
    nc.sync.dma_start(bk_sb, bk.rearrange("(c p) -> p c", p=P))
    ebias_sb = wts.tile([P, 1], F32, tag="ebias")
    nc.vector.memset(ebias_sb, EXP_BIAS)
    # bv broadcast to all 128 partitions, used along free dim of V
    bv_bc = wts.tile([P, GE], F32, tag="bv_bc")
    bv_ap = bv.ap()
    bv_bcast_src = bass.AP(tensor=bv_ap.tensor, offset=bv_ap.offset,
                           ap=[[0, P]] + list(bv_ap.ap))
    nc.gpsimd.dma_start(out=bv_bc, in_=bv_bcast_src)

    # ---- persistent activation tiles ----
    QT_sb = [persist.tile([P, T], BF16, tag=f"QT{c}", name=f"QT{c}") for c in range(2)]
    KT_sb = [persist.tile([P, S], BF16, tag=f"KT{c}", name=f"KT{c}") for c in range(2)]
    # V bf16, head-major: [128, head(4), s-tile(32), 65]; col 64 = 1.0 (den)
    V_sb = persist.tile([P, 4, NSC, 65], BF16, tag="V", name="V")
    nc.vector.memset(V_sb[:, :, :, 64:65], 1.0)
    OcatT = [persist.tile([P, T], BF16, tag=f"Ocat{c}", name=f"Ocat{c}") for c in range(2)]

    ctx8_r = ctx8.rearrange("(c p) s -> p c s", p=P)
    yT_r = yT.rearrange("(m p) t -> p m t", p=P)

    with tc.tile_pool(name="aps", bufs=1, space="PSUM") as aps, \
         tc.tile_pool(name="c8pool", bufs=3) as c8pool, \
         tc.tile_pool(name="espool", bufs=12) as espool, \
         tc.tile_pool(name="npool", bufs=2) as npool, \
         tc.tile_pool(name="dramp", bufs=4, space="DRAM") as dramp, \
         tc.tile_pool(name="ystg", bufs=2) as ystg:

        ctx8_tiles = {}

        def ctx_dma(sg):
            """Fetch ctx half-group sg (512 cols) in fp8."""
            cols = slice(sg * 512, (sg + 1) * 512)
            t8 = c8pool.tile([P, KV_C, 512], FP8, tag="c8")
            nc.sync.dma_start(t8, ctx8_r[:, :, cols])
            ctx8_tiles[sg] = t8

        def k_group(sg):
            """KT[:, sg cols] for both c2 chunks; fp8 DoubleRow; x512."""
            ctx_t = ctx8_tiles[sg]
            ps = aps.tile([P, 1024], F32, tag="slab", bufs=2, name=f"kps{sg}")
            for m in range(2):
                for cp in range(KV_C // 2):
                    nc.tensor.matmul(
                        ps[:, m * 512:(m + 1) * 512],
                        wkT_sb[:, 2 * cp:2 * cp + 2, m * P:(m + 1) * P],
                        ctx_t[:, 2 * cp:2 * cp + 2, :],
                        start=(cp == 0), stop=(cp == KV_C // 2 - 1),
                        perf_mode=DR, skip_group_check=True)
            for m in range(2):
                nc.vector.tensor_scalar_add(
                    KT_sb[m][:, sg * 512:(sg + 1) * 512],
                    ps[:, m * 512:(m + 1) * 512], bk_sb[:, m:m + 1])

        def v_group(sg):
            """V s-tiles of half-group sg; bf16."""
            ctx_t = ctx8_tiles.pop(sg)
            ps = aps.tile([P, 1024], F32, tag="slab", bufs=2, name=f"vps{sg}")
            for st4 in range(SGT):
                for c in range(KV_C):
                    nc.tensor.matmul(
                        ps[:, st4 * 256:(st4 + 1) * 256],
                        ctx_t[:, c, st4 * P:(st4 + 1) * P],
                        wvT_sb[:, c, :],
                        start=(c == 0), stop=(c == KV_C - 1),
                        skip_group_check=True)
            for st4 in range(SGT):
                idx = sg * SGT + st4
                nc.vector.tensor_tensor(
                    V_sb[:, :, idx, 0:64],
                    ps[:, st4 * 256:(st4 + 1) * 256].rearrange(
                        "p (h e) -> p h e", e=64),
                    bv_bc.rearrange("p (h e) -> p h e", e=64),
                    ADD)

        def q_proj():
            for c2 in range(2):
                for tp in range(2):
                    ps = aps.tile([P, 1024], F32, tag="slab", bufs=2,
                                  name=f"qps{c2}{tp}")
                    for tn in range(2):
                        t = tp * 2 + tn
                        for c in range(E_C):
                            nc.tensor.matmul(
                                ps[:, tn * 512:(tn + 1) * 512],
                                wqT_sb[:, c, c2 * P:(c2 + 1) * P],
                                xT_sb[:, c, t * 512:(t + 1) * 512],
                                start=(c == 0), stop=(c == E_C - 1),
                                skip_group_check=True)
                    nc.vector.tensor_scalar_add(
                        QT_sb[c2][:, tp * 1024:(tp + 1) * 1024], ps,
                        bq_sb[:, c2:c2 + 1])

        def attn_unit(c2, t, chase=False, inject=None):
            """Generator: scores + exp + attnV over full S, then normalize.

            When chase=True, yields at every SGT s-tile boundary (before
            the segment that needs fresh K) so the driver can emit K/V
            production; attnV lags one segment (V not yet produced)."""
            tcols = slice(t * 512, (t + 1) * 512)
            o_ps = [aps.tile([P, 512], F32, tag=f"o{j}", name=f"o{c2}{t}{j}",
                             bufs=2) for j in range(2)]
            pending = []

            def emit_attnv(s, es_tile):
                for j in range(2):
                    nc.tensor.matmul(
                        o_ps[j][:65],
                        V_sb[:, 2 * c2 + j, s, 0:65],
                        es_tile[:, j * 512:(j + 1) * 512],
                        start=(s == 0), stop=(s == NSC - 1),
                        skip_group_check=True)

            for s in range(NSC):
                if chase and s % SGT == 0:
                    yield s
                    for (ss, ee) in pending:
                        emit_attnv(ss, ee)
                    pending.clear()
                if inject and s in inject:
                    inject[s]()
                slab = aps.tile([P, 1024], F32, tag="slab", bufs=2,
                                name=f"sl{c2}{t}{s}")
                nc.tensor.matmul(
                    slab[:, 0:512],
                    KT_sb[c2][0:64, s * P:(s + 1) * P],
                    QT_sb[c2][0:64, tcols],
                    start=True, stop=True, skip_group_check=True)
                nc.tensor.matmul(
                    slab[:, 512:1024],
                    KT_sb[c2][64:128, s * P:(s + 1) * P],
                    QT_sb[c2][64:128, tcols],
                    start=True, stop=True, skip_group_check=True)
                es_t = espool.tile([P, 1024], BF16, tag="es", bufs=12)
                nc.scalar.activation(es_t, slab, EXP, scale=SCALE / WK_SCALE,
                                     bias=ebias_sb[:, 0:1])
                if chase:
                    pending.append((s, es_t))
                else:
                    emit_attnv(s, es_t)
            if chase:
                yield NSC
                for (ss, ee) in pending:
                    emit_attnv(ss, ee)
                pending.clear()

            # ---- eviction + normalization ----
            for j in range(2):
                ou = npool.tile([65, 512], F32, tag=f"ou{j}", bufs=2)
                nc.vector.tensor_copy(ou, o_ps[j][:65, :])
                dscr = dramp.tile([1, 512], F32, tag="dscr")
                nc.sync.dma_start(dscr, ou[64:65, :])
                bc = npool.tile([64, 512], F32, tag=f"bc{j}", bufs=2)
                bcast_src = bass.AP(tensor=dscr.tensor, offset=dscr.offset,
                                    ap=[[0, 64]] + list(dscr.ap[1:]))
                nc.gpsimd.dma_start(out=bc, in_=bcast_src)
                iv = npool.tile([64, 512], F32, tag=f"inv{j}", bufs=2)
                nc.vector.reciprocal_approx_fast(out=iv, in_=bc)
                if j == 0:
                    nc.vector.tensor_tensor(
                        OcatT[c2][0:64, tcols], ou[0:64, :], iv, MULT)
                else:
                    stg = npool.tile([64, 512], BF16, tag="stg", bufs=2)
                    nc.vector.tensor_tensor(stg, ou[0:64, :], iv, MULT)
                    nc.sync.dma_start(OcatT[c2][64:128, tcols], stg)

        def y_proj(t, only_mp=None):
            tcols = slice(t * 512, (t + 1) * 512)
            for mp in ((only_mp,) if only_mp is not None else (0, 1)):
                ps = aps.tile([P, 1024], F32, tag="slab", bufs=2, name=f"yps{t}{mp}")
                for mn in range(2):
                    m = mp * 2 + mn
                    for c2 in range(2):
                        nc.tensor.matmul(
                            ps[:, mn * 512:(mn + 1) * 512],
                            woT_sb[:, c2, m * P:(m + 1) * P],
                            OcatT[c2][:, tcols],
                            start=(c2 == 0), stop=(c2 == 1),
                            skip_group_check=True)
                yo = ystg.tile([P, 1024], F32, tag="yo")
                nc.vector.tensor_copy(yo, ps)
                for mn in range(2):
                    m = mp * 2 + mn
                    nc.sync.dma_start(yT_r[:, m, tcols],
                                      yo[:, mn * 512:(mn + 1) * 512])

        # ================= emission schedule =================
        # Chase phase: units (0,0) and (1,0) interleave with K/V production.
        ctx_dma(0)
        k_group(0)
        q_proj()

        u0 = attn_unit(0, 0, chase=True)
        u1 = attn_unit(1, 0, chase=True)
        next(u0)   # emits nothing yet (yield at s=0)
        next(u1)
        for g in range(NSG):
            # produce for the segment the units are about to consume:
            # K(g) already done for g=0; V(g) now; prefetch + K for g+1.
            if g + 1 < NSG:
                ctx_dma(g + 1)
            v_group(g)
            if g + 1 < NSG:
                k_group(g + 1)
            next(u0)   # scores/exp for segment g (+ attnV of segment g-1)
            next(u1)
        for u in (u0, u1):
            try:
                while True:
                    next(u)
            except StopIteration:
                pass

        for t in range(1, NT):
            inj = {4: (lambda tt=t: y_proj(tt - 1, only_mp=0)),
                   12: (lambda tt=t: y_proj(tt - 1, only_mp=1))}
            for _ in attn_unit(0, t, inject=inj):
                pass
            for _ in attn_unit(1, t):
                pass
        y_proj(NT - 1)

    persist.release()
    wts.release()


_NC_CACHE = None
LAST_RESULT = None


def _get_nc():
    global _NC_CACHE
    if _NC_CACHE is None:
        _NC_CACHE = _build_nc()
    return _NC_CACHE


def kernel(x, context, wq_w, wq_b, wk_w, wk_b, wv_w, wv_b, wo_w, wo_b):
    x = np.asarray(x)
    context = np.asarray(context)
    nc = _get_nc()

    ctx8 = [np.ascontiguousarray(context[b].T).astype(NPFP8) for b in range(B)]
    xT = [np.ascontiguousarray(x[b].T).astype(NPBF16) for b in range(B)]

    in_maps = []
    for c in range(N_CORES):
        b, g = c // 2, c % 2
        sl = slice(g * GE, (g + 1) * GE)
        in_maps.append({
            "ctx8": ctx8[b],
            "xT": xT[b],
            "wqT": np.ascontiguousarray(np.asarray(wq_w)[sl, :].T).astype(NPBF16),
            "wkT": np.ascontiguousarray(
                np.asarray(wk_w)[sl, :].T * WK_SCALE).astype(NPFP8),
            "wvT": np.ascontiguousarray(np.asarray(wv_w)[sl, :].T).astype(NPBF16),
            "woT": np.ascontiguousarray(np.asarray(wo_w)[:, sl].T).astype(NPBF16),
            "bq": np.ascontiguousarray(np.asarray(wq_b)[sl]).astype(np.float32),
            "bk": np.ascontiguousarray(
                np.asarray(wk_b)[sl] * WK_SCALE).astype(np.float32),
            "bv": np.ascontiguousarray(np.asarray(wv_b)[sl]).astype(np.float32),
        })

    res = run_bass_kernel_spmd(nc, in_maps, core_ids=list(range(N_CORES)))
    global LAST_RESULT
    LAST_RESULT = res
    outs = res.results

    wo_b = np.asarray(wo_b, dtype=np.float32)
    y = np.empty((B, T, E), dtype=np.float32)
    for b in range(B):
        yt = outs[2 * b]["yT"] + outs[2 * b + 1]["yT"]
        y[b] = yt.T + wo_b
    return y
